# revision 33
# baseline (speedup 1.0000x reference)
"""AUCM loss (pairwise softplus AUC surrogate) Trainium2 kernel.

Reference, for logits/targets [B=1024, C=128]:
    probs = sigmoid(logits)
    num[c] = sum_{i,j} softplus(p_j - p_i) * pos[i,c] * neg[j,c]
    loss   = masked mean over classes of num[c] / (n_pos[c]*n_neg[c])

Direct evaluation is O(B^2 C) = 134M softplus terms.  Since probs in (0,1),
the pairwise argument lies in (-1,1) where softplus is analytic (nearest
complex singularity at +-i*pi), so a degree-6 Chebyshev fit of softplus on
[-1,1] (max err 3.3e-7) turns the pairwise sum into per-class weighted power
sums ("moments") via the binomial expansion:

    num[c] = sum_{m+n<=6} Bm[m,n] * Sn[m,c] * Sp[n,c]
    Sp[n,c] = sum_i pos[i,c] a_i^n,  Sn[m,c] = sum_j neg[j,c] a_j^m

with a_i = tanh(logits_i/2) = 2*(probs_i - 0.5) computed in ONE activation op
(coefficients pre-scaled by 2^-k on the host).  O(B C D) work.

Sharding: data-parallel over the class axis (16 classes/core, batch
replicated, per the pairwise structure).  Each core returns its partial
(sum of per-class means, count of valid classes); the host unshard step sums
the 8 partial pairs and forms the final scalar exactly as the reference does.

Per-core dataflow ([128p, 128f] tile, partition p holds batch rows 8p..8p+7):
  - DVE builds the masked power tiles W_k[p, s, ibc] = mask_s * a^k via 5
    tensor_tensor multiplies (square/product chain).
  - PE does the batch reduction AND the coefficient combination in one
    accumulating matmul group: stationary for moment k is [128, 14] with
    columns j<7 = Bm[j,k] (accumulates H = Bm @ Sp directly) and columns
    j>=7 = one-hot k (collects the raw moments); PSUM [14, 256] accumulates
    over k.
  - One DVE segmented reduce folds the 8-way batch-fold axis: SB [14, 2*16].
  - Tail: G = Sn (.) H, num = ones @ G, per-class mean + validity masking,
    and a [1,2] result (sum of means, valid count) DMA'd out.
"""

import os
import sys
from math import comb

import numpy as np

for _p in ("/opt/trn_rl_repo", "/root/.axon_site/_ro/trn_rl_repo"):
    if os.path.isdir(_p) and _p not in sys.path:
        sys.path.append(_p)

import ml_dtypes

import concourse.bacc as bacc
import concourse.bass as bass
import concourse.mybir as mybir
import concourse.tile as tile
from concourse import bass_utils

B_FULL, C_FULL = 1024, 128
N_CORES = 8
C_SHARD = C_FULL // N_CORES          # 16 classes per core
P = 128                              # partitions
IB = B_FULL // P                     # 8 batch rows folded per partition
DEG = 6
NMOM = DEG + 1                       # 7 moments (k = 0..6)
NST = 2 * NMOM                       # stationary columns (H part + raw part)
ONES_COL = NMOM * NST                # all-ones column (final sum lhsT)
SEL_COL = ONES_COL + 1               # 7-wide row-selection block (rows 7..13)
CN_COLS = SEL_COL + NMOM + 1         # + pad
BF_K = (3, 4, 5, 6)                  # moments with bf16 data + stationaries

# Degree-6 Chebyshev fit of softplus on [-1, 1] (max err 3.3e-7), monomial.
A_COEF = np.array(
    [0.6931471805599451, 0.5, 0.12499748720039783, 0.0,
     -0.005188028447445448, 0.0, 0.0003053804886608954],
    dtype=np.float64,
)


def _host_consts():
    # moments are of a = tanh(x/2) = 2*(p - 0.5); rescale poly coeffs by 2^-k
    alpha = A_COEF / (2.0 ** np.arange(NMOM))
    bm = np.zeros((NMOM, NMOM))
    for m in range(NMOM):
        for n in range(NMOM - m):
            bm[m, n] = alpha[m + n] * comb(m + n, m) * ((-1.0) ** n)
    row = np.zeros(CN_COLS, np.float32)
    for k in range(NMOM):
        row[k * NST:k * NST + NMOM] = bm[:, k]       # H-part: col j = Bm[j, k]
        row[k * NST + NMOM + k] = 1.0                # raw part: one-hot k
    row[ONES_COL] = 1.0                              # ones column (final sum)
    cn = np.ascontiguousarray(np.broadcast_to(row, (P, CN_COLS)), np.float32)
    # row-selection block: lhsT [14, 7] picking rows 7..13 down to 0..6
    for m in range(NMOM):
        cn[NMOM + m, SEL_COL + m] = 1.0
    # bf16 stationaries for the small-contribution moments k in BF_K
    rowb = np.zeros((len(BF_K), NST), np.float32)
    for i, k in enumerate(BF_K):
        rowb[i, :NMOM] = bm[:, k]
        rowb[i, NMOM + k] = 1.0
    cnb = np.ascontiguousarray(
        np.broadcast_to(rowb.reshape(1, -1), (P, len(BF_K) * NST))
    ).astype(ml_dtypes.bfloat16)
    return cn, cnb


def build_bass():
    f32 = mybir.dt.float32
    nc = bacc.Bacc("TRN2", target_bir_lowering=False, debug=False)

    bf = mybir.dt.bfloat16
    lg = nc.dram_tensor("logits", [B_FULL, C_SHARD], f32, kind="ExternalInput")
    tg = nc.dram_tensor("targets", [B_FULL, C_SHARD], f32, kind="ExternalInput")
    cn = nc.dram_tensor("cn", [P, CN_COLS], f32, kind="ExternalInput")
    cnb = nc.dram_tensor("cnb", [P, len(BF_K) * NST], bf, kind="ExternalInput")
    out_d = nc.dram_tensor("out", [1, 2], f32, kind="ExternalOutput")

    mult = mybir.AluOpType.mult
    add = mybir.AluOpType.add
    is_gt = mybir.AluOpType.is_gt

    with tile.TileContext(nc) as tc:
        with (
            tc.tile_pool(name="sb", bufs=1) as pool,
            tc.tile_pool(name="ps", bufs=1, space="PSUM") as pps,
        ):
            # ---- PE warmup: dummy matmuls during the input-DMA window so
            # the HAM clock gate is at 2.4 GHz when the real matmuls arrive.
            # Their PSUM tile is never read; contents are irrelevant.
            WU = pool.tile([P, 256], mybir.dt.bfloat16, tag="WU")
            nc.gpsimd.memset(WU[:, :], 0.0)
            DPS = pps.tile([1, 256], f32, tag="DPS")
            for _ in range(15):
                nc.tensor.matmul(DPS[:, :], WU[:, 0:1], WU[:, :],
                                 start=True, stop=True)

            # ---- inputs -> SBUF (contiguous loads, two HWDGE rings) --------
            # targets land directly in W0's pos half (saves a copy op)
            shp = [P, 2, IB * C_SHARD]
            W0 = pool.tile(shp, f32, tag="W0")
            X = pool.tile([P, IB * C_SHARD], f32, tag="X")
            CN = pool.tile([P, CN_COLS], f32, tag="CN")
            nc.sync.dma_start(
                out=X[:, :], in_=lg.ap().rearrange("(p q) c -> p (q c)", p=P)
            )
            nc.scalar.dma_start(
                out=W0[:, 0, :], in_=tg.ap().rearrange("(p q) c -> p (q c)", p=P)
            )
            nc.sync.dma_start(out=CN[:, :], in_=cn.ap())
            CNB = pool.tile([P, len(BF_K) * NST], bf, tag="CNB")
            nc.sync.dma_start(out=CNB[:, :], in_=cnb.ap())

            # ---- a = tanh(x/2) ---------------------------------------------
            A = pool.tile([P, IB * C_SHARD], f32, tag="A")
            nc.scalar.activation(
                A[:, :], X[:, :], mybir.ActivationFunctionType.Tanh, scale=0.5
            )

            # ---- masked power tiles W_k[p, s, ibc] = mask_s * a^k ----------
            # k = 0..2 in fp32 (dominant coefficients), k = 3..6 in bf16
            # (tiny contributions -> rounding is far below fp32 noise floor).
            # The two squares run on the otherwise-idle scalar engine.
            W1 = pool.tile(shp, f32, tag="W1")
            W2 = pool.tile(shp, f32, tag="W2")
            W3b = pool.tile(shp, bf, tag="W3b")
            W4b = pool.tile(shp, bf, tag="W4b")
            W5b = pool.tile(shp, bf, tag="W5b")
            W6b = pool.tile(shp, bf, tag="W6b")
            nc.vector.tensor_scalar(W0[:, 1, :], W0[:, 0, :], -1.0, 1.0,
                                    op0=mult, op1=add)                  # 1 - t
            nc.vector.tensor_mul(W1[:, 0, :], W0[:, 0, :], A[:, :])
            nc.vector.tensor_mul(W1[:, 1, :], W0[:, 1, :], A[:, :])
            nc.vector.tensor_mul(W2[:, :, :], W1[:, :, :], W1[:, :, :])
            nc.vector.tensor_mul(W3b[:, :, :], W1[:, :, :], W2[:, :, :])
            nc.vector.tensor_mul(W6b[:, :, :], W3b[:, :, :], W3b[:, :, :])
            nc.scalar.activation(W4b[:, :, :], W2[:, :, :],
                                 mybir.ActivationFunctionType.Square)
            nc.vector.tensor_mul(W5b[:, :, :], W2[:, :, :], W3b[:, :, :])

            # ---- PE: batch-sum + coefficient combination, one MM group ----
            # PSA[j, (s ib c)] = sum_k ST[k][j] * colsum_p(W_k)
            PSA = pps.tile([NST, 2 * IB * C_SHARD], f32, tag="PSA")
            mm_plan = [
                (W0, CN[:, 0 * NST:1 * NST]),
                (W1, CN[:, 1 * NST:2 * NST]),
                (W2, CN[:, 2 * NST:3 * NST]),
                (W3b, CNB[:, 0 * NST:1 * NST]),
                (W6b, CNB[:, 3 * NST:4 * NST]),
                (W4b, CNB[:, 1 * NST:2 * NST]),
                (W5b, CNB[:, 2 * NST:3 * NST]),
            ]
            for k, (wk, lhsT) in enumerate(mm_plan):
                nc.tensor.matmul(
                    PSA[:, :], lhsT, wk[:, :, :],
                    start=(k == 0), stop=(k == NMOM - 1),
                )

            # ---- fold the ib axis: SB[j, s*16+c] ---------------------------
            SB = pool.tile([NST, 2 * C_SHARD], f32, tag="SB")
            nc.vector.reduce_sum(
                SB[:, :].rearrange("p (s c) -> p s c", s=2),
                PSA[:, :].rearrange("p (s ib c) -> p s c ib", s=2, ib=IB),
                axis=mybir.AxisListType.X,
            )
            # rows 0..6 cols 0:16   = H[m,c] = sum_n Bm[m,n] Sp[n,c]
            # rows 7..13            = [Sp[k] | Sn[k]]

            # ---- relocate raw rows 7..13 to partitions 0..6 (matmul moves
            # partitions; DVE slices must start at partition 0/32/64/96) ----
            RAW = pps.tile([NMOM, 2 * C_SHARD], f32, tag="RAW")
            nc.tensor.matmul(
                RAW[:, :], CN[0:NST, SEL_COL:SEL_COL + NMOM], SB[:, :],
                start=True, stop=True,
            )

            # ---- num[c] = sum_m Sn[m,c] * H[m,c] ---------------------------
            G = pool.tile([NMOM, C_SHARD], f32, tag="G")
            nc.vector.tensor_mul(
                G[:, :], RAW[:, C_SHARD:2 * C_SHARD], SB[0:NMOM, 0:C_SHARD]
            )
            NUM = pps.tile([1, C_SHARD], f32, tag="NUM")
            nc.tensor.matmul(
                NUM[:, :], CN[0:NMOM, ONES_COL:ONES_COL + 1], G[:, :],
                start=True, stop=True,
            )

            # ---- per-class mean + validity ---------------------------------
            # n_neg = B - n_pos exactly; work with cntneg = (Sp0 - B)*Sp0 =
            # -cnt so each op reads PSUM at most once (no two-PSUM operands).
            # The validity channel is stored negated; the host flips the sign.
            RES = pool.tile([1, 2, C_SHARD], f32, tag="RES")
            nneg = pool.tile([1, C_SHARD], f32, tag="nneg")
            nc.vector.tensor_scalar(nneg[:, :], RAW[0:1, 0:C_SHARD],
                                    -1.0, float(B_FULL), op0=mult, op1=add)
            cnt = pool.tile([1, C_SHARD], f32, tag="cnt")
            nc.vector.tensor_mul(cnt[:, :], nneg[:, :], RAW[0:1, 0:C_SHARD])
            nc.vector.tensor_scalar(RES[:, 1, :], cnt[:, :], 0.5, None, op0=is_gt)
            safe = pool.tile([1, C_SHARD], f32, tag="safe")
            nc.vector.tensor_scalar_max(safe[:, :], cnt[:, :], 1.0)
            rec = pool.tile([1, C_SHARD], f32, tag="rec")
            nc.vector.reciprocal(rec[:, :], safe[:, :])
            mask2 = pool.tile([1, C_SHARD], f32, tag="mask2")
            nc.vector.tensor_mul(mask2[:, :], rec[:, :], RES[:, 1, :])
            nc.vector.tensor_mul(RES[:, 0, :], NUM[:, :], mask2[:, :])

            OUT = pool.tile([1, 2], f32, tag="OUT")
            nc.vector.reduce_sum(OUT[:, :], RES[:, :, :], axis=mybir.AxisListType.X)
            nc.sync.dma_start(out=out_d.ap(), in_=OUT[:, :])

    nc.compile()
    return nc


_CACHE = {}


def _compiled():
    if "nc" not in _CACHE:
        _CACHE["nc"] = build_bass()
    return _CACHE["nc"]


def make_in_maps(logits, targets):
    cn, cnb = _host_consts()
    logits = np.ascontiguousarray(logits, dtype=np.float32)
    targets = np.ascontiguousarray(targets, dtype=np.float32)
    in_maps = []
    for k in range(N_CORES):
        sl = slice(k * C_SHARD, (k + 1) * C_SHARD)
        in_maps.append({
            "logits": np.ascontiguousarray(logits[:, sl]),
            "targets": np.ascontiguousarray(targets[:, sl]),
            "cn": cn,
            "cnb": cnb,
        })
    return in_maps


def combine_outputs(core_outs):
    """core_outs: list of [1,2] arrays -> scalar loss (matches reference)."""
    f32 = np.float32
    parts = np.stack([np.asarray(o, f32).reshape(2) for o in core_outs])
    sums = parts[:, 0].sum(dtype=f32)
    vc = parts[:, 1].sum(dtype=f32)
    if vc > 0:
        loss = f32(sums / max(vc, f32(1.0)))
    else:
        loss = f32(0.0)
    return np.asarray(loss, dtype=np.float32)


def kernel(logits, targets):
    nc = _compiled()
    in_maps = make_in_maps(logits, targets)
    res = bass_utils.run_bass_kernel_spmd(nc, in_maps, core_ids=list(range(N_CORES)))
    return combine_outputs([r["out"] for r in res.results])


# revision 36
# speedup vs baseline: 1.1998x; 1.1998x over previous
"""AUCM loss (pairwise softplus AUC surrogate) Trainium2 kernel.

Reference, for logits/targets [B=1024, C=128]:
    probs = sigmoid(logits)
    num[c] = sum_{i,j} softplus(p_j - p_i) * pos[i,c] * neg[j,c]
    loss   = masked mean over classes of num[c] / (n_pos[c]*n_neg[c])

Direct evaluation is O(B^2 C) = 134M softplus terms.  Since probs in (0,1),
the pairwise argument lies in (-1,1) where softplus is analytic (nearest
complex singularity at +-i*pi), so a degree-6 Chebyshev fit of softplus on
[-1,1] (max err 3.3e-7) turns the pairwise sum into per-class weighted power
sums ("moments") via the binomial expansion:

    num[c] = sum_{m+n<=6} Bm[m,n] * Sn[m,c] * Sp[n,c]
    Sp[n,c] = sum_i pos[i,c] a_i^n,  Sn[m,c] = sum_j neg[j,c] a_j^m

with a_i = tanh(logits_i/2) = 2*(probs_i - 0.5) computed in ONE activation op
(coefficients pre-scaled by 2^-k on the host).  O(B C D) work.

Sharding: data-parallel over the class axis (16 classes/core, batch
replicated, per the pairwise structure).  Each core returns its partial
(sum of per-class means, count of valid classes); the host unshard step sums
the 8 partial pairs and forms the final scalar exactly as the reference does.

Per-core dataflow ([128p, 128f] tile, partition p holds batch rows 8p..8p+7):
  - DVE builds the masked power tiles W_k[p, s, ibc] = mask_s * a^k via 5
    tensor_tensor multiplies (square/product chain).
  - PE does the batch reduction AND the coefficient combination in one
    accumulating matmul group: stationary for moment k is [128, 14] with
    columns j<7 = Bm[j,k] (accumulates H = Bm @ Sp directly) and columns
    j>=7 = one-hot k (collects the raw moments); PSUM [14, 256] accumulates
    over k.
  - One DVE segmented reduce folds the 8-way batch-fold axis: SB [14, 2*16].
  - Tail: G = Sn (.) H, num = ones @ G, per-class mean + validity masking,
    and a [1,2] result (sum of means, valid count) DMA'd out.
"""

import os
import sys
from math import comb

import numpy as np

for _p in ("/opt/trn_rl_repo", "/root/.axon_site/_ro/trn_rl_repo"):
    if os.path.isdir(_p) and _p not in sys.path:
        sys.path.append(_p)

import ml_dtypes

import concourse.bacc as bacc
import concourse.bass as bass
import concourse.mybir as mybir
import concourse.tile as tile
from concourse import bass_utils

B_FULL, C_FULL = 1024, 128
N_CORES = 8
C_SHARD = C_FULL // N_CORES          # 16 classes per core
P = 128                              # partitions
IB = B_FULL // P                     # 8 batch rows folded per partition
DEG = 6
NMOM = DEG + 1                       # 7 moments (k = 0..6)
NST = 2 * NMOM                       # stationary columns (H part + raw part)
ONES_COL = NMOM * NST                # all-ones column (final sum lhsT)
SEL_COL = ONES_COL + 1               # 7-wide row-selection block (rows 7..13)
CN_COLS = SEL_COL + NMOM + 1         # + pad
BF_K = (3, 4, 5, 6)                  # moments with bf16 data + stationaries

# Degree-6 Chebyshev fit of softplus on [-1, 1] (max err 3.3e-7), monomial.
A_COEF = np.array(
    [0.6931471805599451, 0.5, 0.12499748720039783, 0.0,
     -0.005188028447445448, 0.0, 0.0003053804886608954],
    dtype=np.float64,
)


def _host_consts():
    # moments are of a = tanh(x/2) = 2*(p - 0.5); rescale poly coeffs by 2^-k
    alpha = A_COEF / (2.0 ** np.arange(NMOM))
    bm = np.zeros((NMOM, NMOM))
    for m in range(NMOM):
        for n in range(NMOM - m):
            bm[m, n] = alpha[m + n] * comb(m + n, m) * ((-1.0) ** n)
    row = np.zeros(CN_COLS, np.float32)
    for k in range(NMOM):
        row[k * NST:k * NST + NMOM] = bm[:, k]       # H-part: col j = Bm[j, k]
        row[k * NST + NMOM + k] = 1.0                # raw part: one-hot k
    row[ONES_COL] = 1.0                              # ones column (final sum)
    cn = np.ascontiguousarray(np.broadcast_to(row, (P, CN_COLS)), np.float32)
    # row-selection block: lhsT [14, 7] picking rows 7..13 down to 0..6
    for m in range(NMOM):
        cn[NMOM + m, SEL_COL + m] = 1.0
    # bf16 stationaries for the small-contribution moments k in BF_K
    rowb = np.zeros((len(BF_K), NST), np.float32)
    for i, k in enumerate(BF_K):
        rowb[i, :NMOM] = bm[:, k]
        rowb[i, NMOM + k] = 1.0
    cnb = np.ascontiguousarray(
        np.broadcast_to(rowb.reshape(1, -1), (P, len(BF_K) * NST))
    ).astype(ml_dtypes.bfloat16)
    return cn, cnb


def build_bass():
    f32 = mybir.dt.float32
    nc = bacc.Bacc("TRN2", target_bir_lowering=False, debug=False)

    bf = mybir.dt.bfloat16
    lg = nc.dram_tensor("logits", [B_FULL, C_SHARD], f32, kind="ExternalInput")
    tg = nc.dram_tensor("targets", [B_FULL, C_SHARD], f32, kind="ExternalInput")
    cn = nc.dram_tensor("cn", [P, CN_COLS], f32, kind="ExternalInput")
    cnb = nc.dram_tensor("cnb", [P, len(BF_K) * NST], bf, kind="ExternalInput")
    out_d = nc.dram_tensor("out", [1, 2], f32, kind="ExternalOutput")

    mult = mybir.AluOpType.mult
    add = mybir.AluOpType.add
    is_gt = mybir.AluOpType.is_gt

    with tile.TileContext(nc) as tc:
        with (
            tc.tile_pool(name="sb", bufs=1) as pool,
            tc.tile_pool(name="ps", bufs=1, space="PSUM") as pps,
        ):
            # ---- PE warmup: dummy matmuls during the input-DMA window so
            # the HAM clock gate is at 2.4 GHz when the real matmuls arrive.
            # Their PSUM tile is never read; contents are irrelevant.
            WU = pool.tile([P, 256], mybir.dt.bfloat16, tag="WU")
            nc.gpsimd.memset(WU[:, :], 0.0)
            DPS = pps.tile([1, 256], f32, tag="DPS")
            for _ in range(15):
                nc.tensor.matmul(DPS[:, :], WU[:, 0:1], WU[:, :],
                                 start=True, stop=True)

            # ---- inputs -> SBUF (contiguous loads, two HWDGE rings) --------
            # targets land directly in W0's pos half (saves a copy op)
            shp = [P, 2, IB * C_SHARD]
            W0 = pool.tile(shp, f32, tag="W0")
            X = pool.tile([P, IB * C_SHARD], f32, tag="X")
            CN = pool.tile([P, CN_COLS], f32, tag="CN")
            nc.sync.dma_start(
                out=X[:, :], in_=lg.ap().rearrange("(p q) c -> p (q c)", p=P)
            )
            nc.scalar.dma_start(
                out=W0[:, 0, :], in_=tg.ap().rearrange("(p q) c -> p (q c)", p=P)
            )
            nc.sync.dma_start(out=CN[:, :], in_=cn.ap())
            CNB = pool.tile([P, len(BF_K) * NST], bf, tag="CNB")
            nc.sync.dma_start(out=CNB[:, :], in_=cnb.ap())

            # ---- a = tanh(x/2) ---------------------------------------------
            A = pool.tile([P, IB * C_SHARD], f32, tag="A")
            nc.scalar.activation(
                A[:, :], X[:, :], mybir.ActivationFunctionType.Tanh, scale=0.5
            )

            # ---- masked power tiles W_k[p, s, ibc] = mask_s * a^k ----------
            # k = 0..2 in fp32 (dominant coefficients), k = 3..6 in bf16
            # (tiny contributions -> rounding is far below fp32 noise floor).
            # The two squares run on the otherwise-idle scalar engine.
            W1 = pool.tile(shp, f32, tag="W1")
            W2 = pool.tile(shp, f32, tag="W2")
            W3b = pool.tile(shp, bf, tag="W3b")
            W4b = pool.tile(shp, bf, tag="W4b")
            W5b = pool.tile(shp, bf, tag="W5b")
            W6b = pool.tile(shp, bf, tag="W6b")
            nc.vector.tensor_scalar(W0[:, 1, :], W0[:, 0, :], -1.0, 1.0,
                                    op0=mult, op1=add)                  # 1 - t
            nc.vector.tensor_mul(W1[:, 0, :], W0[:, 0, :], A[:, :])
            nc.vector.tensor_mul(W1[:, 1, :], W0[:, 1, :], A[:, :])
            nc.vector.tensor_mul(W2[:, :, :], W1[:, :, :], W1[:, :, :])
            nc.vector.tensor_mul(W3b[:, :, :], W1[:, :, :], W2[:, :, :])
            nc.vector.tensor_mul(W6b[:, :, :], W3b[:, :, :], W3b[:, :, :])
            nc.scalar.activation(W4b[:, :, :], W2[:, :, :],
                                 mybir.ActivationFunctionType.Square)
            nc.vector.tensor_mul(W5b[:, :, :], W2[:, :, :], W3b[:, :, :])

            # ---- PE: batch-sum + coefficient combination -------------------
            # PSA [14, 256] accumulates the wide moments (k=0 fp32, k=3..6
            # bf16).  k=1,2 are ib-prefolded on DVE after the chain (R1/R2,
            # [128, 32]) so their fp32 LOW/HIGH matmuls stream 32 columns
            # instead of 256; they accumulate in PSB [14, 32].
            PSA = pps.tile([NST, 2 * IB * C_SHARD], f32, tag="PSA")
            PSB = pps.tile([NST, 2 * C_SHARD], f32, tag="PSB")
            mm_plan = [
                (W0, CN[:, 0 * NST:1 * NST]),
                (W3b, CNB[:, 0 * NST:1 * NST]),
                (W6b, CNB[:, 3 * NST:4 * NST]),
                (W4b, CNB[:, 1 * NST:2 * NST]),
                (W5b, CNB[:, 2 * NST:3 * NST]),
            ]
            for k, (wk, lhsT) in enumerate(mm_plan):
                nc.tensor.matmul(
                    PSA[:, :], lhsT, wk[:, :, :],
                    start=(k == 0), stop=(k == len(mm_plan) - 1),
                )
            R1 = pool.tile([P, 2 * C_SHARD], f32, tag="R1")
            R2 = pool.tile([P, 2 * C_SHARD], f32, tag="R2")
            nc.vector.reduce_sum(
                R1[:, :].rearrange("p (s c) -> p s c", s=2),
                W1[:, :, :].rearrange("p s (ib c) -> p s c ib", ib=IB),
                axis=mybir.AxisListType.X,
            )
            nc.vector.reduce_sum(
                R2[:, :].rearrange("p (s c) -> p s c", s=2),
                W2[:, :, :].rearrange("p s (ib c) -> p s c ib", ib=IB),
                axis=mybir.AxisListType.X,
            )
            nc.tensor.matmul(PSB[:, :], CN[:, 1 * NST:2 * NST], R1[:, :],
                             start=True, stop=False)
            nc.tensor.matmul(PSB[:, :], CN[:, 2 * NST:3 * NST], R2[:, :],
                             start=False, stop=True)

            # ---- fold the ib axis of PSA, merge PSB: SBF[j, s*16+c] --------
            SB = pool.tile([NST, 2 * C_SHARD], f32, tag="SB")
            nc.vector.reduce_sum(
                SB[:, :].rearrange("p (s c) -> p s c", s=2),
                PSA[:, :].rearrange("p (s ib c) -> p s c ib", s=2, ib=IB),
                axis=mybir.AxisListType.X,
            )
            SBF = pool.tile([NST, 2 * C_SHARD], f32, tag="SBF")
            nc.vector.tensor_add(SBF[:, :], SB[:, :], PSB[:, :])
            # rows 0..6 cols 0:16   = H[m,c] = sum_n Bm[m,n] Sp[n,c]
            # rows 7..13            = [Sp[k] | Sn[k]]

            # ---- relocate raw rows 7..13 to partitions 0..6 (matmul moves
            # partitions; DVE slices must start at partition 0/32/64/96) ----
            RAW = pps.tile([NMOM, 2 * C_SHARD], f32, tag="RAW")
            nc.tensor.matmul(
                RAW[:, :], CN[0:NST, SEL_COL:SEL_COL + NMOM], SBF[:, :],
                start=True, stop=True,
            )

            # ---- num[c] = sum_m Sn[m,c] * H[m,c] ---------------------------
            G = pool.tile([NMOM, C_SHARD], f32, tag="G")
            nc.vector.tensor_mul(
                G[:, :], RAW[:, C_SHARD:2 * C_SHARD], SBF[0:NMOM, 0:C_SHARD]
            )
            NUM = pps.tile([1, C_SHARD], f32, tag="NUM")
            nc.tensor.matmul(
                NUM[:, :], CN[0:NMOM, ONES_COL:ONES_COL + 1], G[:, :],
                start=True, stop=True,
            )

            # ---- per-class mean + validity ---------------------------------
            # n_neg = B - n_pos exactly; work with cntneg = (Sp0 - B)*Sp0 =
            # -cnt so each op reads PSUM at most once (no two-PSUM operands).
            # The validity channel is stored negated; the host flips the sign.
            RES = pool.tile([1, 2, C_SHARD], f32, tag="RES")
            nneg = pool.tile([1, C_SHARD], f32, tag="nneg")
            nc.vector.tensor_scalar(nneg[:, :], RAW[0:1, 0:C_SHARD],
                                    -1.0, float(B_FULL), op0=mult, op1=add)
            cnt = pool.tile([1, C_SHARD], f32, tag="cnt")
            nc.vector.tensor_mul(cnt[:, :], nneg[:, :], RAW[0:1, 0:C_SHARD])
            nc.vector.tensor_scalar(RES[:, 1, :], cnt[:, :], 0.5, None, op0=is_gt)
            safe = pool.tile([1, C_SHARD], f32, tag="safe")
            nc.vector.tensor_scalar_max(safe[:, :], cnt[:, :], 1.0)
            rec = pool.tile([1, C_SHARD], f32, tag="rec")
            nc.vector.reciprocal(rec[:, :], safe[:, :])
            mask2 = pool.tile([1, C_SHARD], f32, tag="mask2")
            nc.vector.tensor_mul(mask2[:, :], rec[:, :], RES[:, 1, :])
            nc.vector.tensor_mul(RES[:, 0, :], NUM[:, :], mask2[:, :])

            OUT = pool.tile([1, 2], f32, tag="OUT")
            nc.vector.reduce_sum(OUT[:, :], RES[:, :, :], axis=mybir.AxisListType.X)
            nc.sync.dma_start(out=out_d.ap(), in_=OUT[:, :])

    nc.compile()
    return nc


_CACHE = {}


def _compiled():
    if "nc" not in _CACHE:
        _CACHE["nc"] = build_bass()
    return _CACHE["nc"]


def make_in_maps(logits, targets):
    cn, cnb = _host_consts()
    logits = np.ascontiguousarray(logits, dtype=np.float32)
    targets = np.ascontiguousarray(targets, dtype=np.float32)
    in_maps = []
    for k in range(N_CORES):
        sl = slice(k * C_SHARD, (k + 1) * C_SHARD)
        in_maps.append({
            "logits": np.ascontiguousarray(logits[:, sl]),
            "targets": np.ascontiguousarray(targets[:, sl]),
            "cn": cn,
            "cnb": cnb,
        })
    return in_maps


def combine_outputs(core_outs):
    """core_outs: list of [1,2] arrays -> scalar loss (matches reference)."""
    f32 = np.float32
    parts = np.stack([np.asarray(o, f32).reshape(2) for o in core_outs])
    sums = parts[:, 0].sum(dtype=f32)
    vc = parts[:, 1].sum(dtype=f32)
    if vc > 0:
        loss = f32(sums / max(vc, f32(1.0)))
    else:
        loss = f32(0.0)
    return np.asarray(loss, dtype=np.float32)


def kernel(logits, targets):
    nc = _compiled()
    in_maps = make_in_maps(logits, targets)
    res = bass_utils.run_bass_kernel_spmd(nc, in_maps, core_ids=list(range(N_CORES)))
    return combine_outputs([r["out"] for r in res.results])


# revision 38
# speedup vs baseline: 1.2053x; 1.0046x over previous
"""AUCM loss (pairwise softplus AUC surrogate) Trainium2 kernel.

Reference, for logits/targets [B=1024, C=128]:
    probs = sigmoid(logits)
    num[c] = sum_{i,j} softplus(p_j - p_i) * pos[i,c] * neg[j,c]
    loss   = masked mean over classes of num[c] / (n_pos[c]*n_neg[c])

Direct evaluation is O(B^2 C) = 134M softplus terms.  Since probs in (0,1),
the pairwise argument lies in (-1,1) where softplus is analytic (nearest
complex singularity at +-i*pi), so a degree-6 Chebyshev fit of softplus on
[-1,1] (max err 3.3e-7) turns the pairwise sum into per-class weighted power
sums ("moments") via the binomial expansion:

    num[c] = sum_{m+n<=6} Bm[m,n] * Sn[m,c] * Sp[n,c]
    Sp[n,c] = sum_i pos[i,c] a_i^n,  Sn[m,c] = sum_j neg[j,c] a_j^m

with a_i = tanh(logits_i/2) = 2*(probs_i - 0.5) computed in ONE activation op
(coefficients pre-scaled by 2^-k on the host).  O(B C D) work.

Sharding: data-parallel over the class axis (16 classes/core, batch
replicated, per the pairwise structure).  Each core returns its partial
(sum of per-class means, count of valid classes); the host unshard step sums
the 8 partial pairs and forms the final scalar exactly as the reference does.

Per-core dataflow ([128p, 128f] tile, partition p holds batch rows 8p..8p+7):
  - DVE builds the masked power tiles W_k[p, s, ibc] = mask_s * a^k via a
    square/product chain; k=0..2 in fp32 (dominant coefficients), k=3..6 in
    bf16 (tiny contributions); one square runs on the scalar engine.
  - PE does the batch reduction AND the coefficient combination in
    accumulating matmul groups: the stationary for moment k is [128, 14]
    with columns j<7 = Bm[j,k] (accumulates H = Bm @ Sp directly) and
    columns j>=7 = one-hot k (collects raw moments).  k=1,2 are ib-prefolded
    on DVE so their fp32 LOW/HIGH matmuls stream 32 columns instead of 256.
    Dummy warmup matmuls during the input-DMA window keep the PE HAM clock
    gate open.
  - One DVE segmented reduce folds the 8-way batch axis, a tiny selection
    matmul relocates raw-moment rows to partition 0 (DVE slices must start
    at partition 0/32/64/96).
  - Tail: G = Sn (.) H, num = ones @ G, per-class mean + validity masking,
    and a [1,2] result (sum of means, valid count) DMA'd out.
"""

import os
import sys
from math import comb

import numpy as np

for _p in ("/opt/trn_rl_repo", "/root/.axon_site/_ro/trn_rl_repo"):
    if os.path.isdir(_p) and _p not in sys.path:
        sys.path.append(_p)

import ml_dtypes

import concourse.bacc as bacc
import concourse.mybir as mybir
import concourse.tile as tile
from concourse import bass_utils

B_FULL, C_FULL = 1024, 128
N_CORES = 8
C_SHARD = C_FULL // N_CORES          # 16 classes per core
P = 128                              # partitions
IB = B_FULL // P                     # 8 batch rows folded per partition
DEG = 6
NMOM = DEG + 1                       # 7 moments (k = 0..6)
NST = 2 * NMOM                       # stationary columns (H part + raw part)
ONES_COL = NMOM * NST                # all-ones column (final sum lhsT)
SEL_COL = ONES_COL + 1               # 7-wide row-selection block (rows 7..13)
CN_COLS = SEL_COL + NMOM + 1         # + pad
BF_K = (3, 4, 5, 6)                  # moments with bf16 data + stationaries

# Degree-6 Chebyshev fit of softplus on [-1, 1] (max err 3.3e-7), monomial.
A_COEF = np.array(
    [0.6931471805599451, 0.5, 0.12499748720039783, 0.0,
     -0.005188028447445448, 0.0, 0.0003053804886608954],
    dtype=np.float64,
)


def _host_consts():
    # moments are of a = tanh(x/2) = 2*(p - 0.5); rescale poly coeffs by 2^-k
    alpha = A_COEF / (2.0 ** np.arange(NMOM))
    bm = np.zeros((NMOM, NMOM))
    for m in range(NMOM):
        for n in range(NMOM - m):
            bm[m, n] = alpha[m + n] * comb(m + n, m) * ((-1.0) ** n)
    row = np.zeros(CN_COLS, np.float32)
    for k in range(NMOM):
        row[k * NST:k * NST + NMOM] = bm[:, k]       # H-part: col j = Bm[j, k]
        row[k * NST + NMOM + k] = 1.0                # raw part: one-hot k
    row[ONES_COL] = 1.0                              # ones column (final sum)
    cn = np.ascontiguousarray(np.broadcast_to(row, (P, CN_COLS)), np.float32)
    # row-selection block: lhsT [14, 7] picking rows 7..13 down to 0..6
    for m in range(NMOM):
        cn[NMOM + m, SEL_COL + m] = 1.0
    # bf16 stationaries for the small-contribution moments k in BF_K
    rowb = np.zeros((len(BF_K), NST), np.float32)
    for i, k in enumerate(BF_K):
        rowb[i, :NMOM] = bm[:, k]
        rowb[i, NMOM + k] = 1.0
    cnb = np.ascontiguousarray(
        np.broadcast_to(rowb.reshape(1, -1), (P, len(BF_K) * NST))
    ).astype(ml_dtypes.bfloat16)
    return cn, cnb


def build_bass():
    f32 = mybir.dt.float32
    nc = bacc.Bacc("TRN2", target_bir_lowering=False, debug=False)

    bf = mybir.dt.bfloat16
    lg = nc.dram_tensor("logits", [B_FULL, C_SHARD], f32, kind="ExternalInput")
    tg = nc.dram_tensor("targets", [B_FULL, C_SHARD], f32, kind="ExternalInput")
    cn = nc.dram_tensor("cn", [P, CN_COLS], f32, kind="ExternalInput")
    cnb = nc.dram_tensor("cnb", [P, len(BF_K) * NST], bf, kind="ExternalInput")
    out_d = nc.dram_tensor("out", [1, 2], f32, kind="ExternalOutput")

    mult = mybir.AluOpType.mult
    add = mybir.AluOpType.add
    is_gt = mybir.AluOpType.is_gt

    with tile.TileContext(nc) as tc:
        with (
            tc.tile_pool(name="sb", bufs=1) as pool,
            tc.tile_pool(name="ps", bufs=1, space="PSUM") as pps,
        ):
            # ---- PE warmup: dummy matmuls during the input-DMA window so
            # the HAM clock gate is at 2.4 GHz when the real matmuls arrive.
            # Their PSUM tile is never read; contents are irrelevant.
            WU = pool.tile([P, 256], mybir.dt.bfloat16, tag="WU")
            nc.gpsimd.memset(WU[:, :], 0.0)
            DPS = pps.tile([1, 256], f32, tag="DPS")
            for _ in range(15):
                nc.tensor.matmul(DPS[:, :], WU[:, 0:1], WU[:, :],
                                 start=True, stop=True)

            # ---- inputs -> SBUF (contiguous loads, two HWDGE rings) --------
            # targets land directly in W0's pos half (saves a copy op)
            shp = [P, 2, IB * C_SHARD]
            W0 = pool.tile(shp, f32, tag="W0")
            X = pool.tile([P, IB * C_SHARD], f32, tag="X")
            CN = pool.tile([P, CN_COLS], f32, tag="CN")
            nc.sync.dma_start(
                out=X[:, :], in_=lg.ap().rearrange("(p q) c -> p (q c)", p=P)
            )
            nc.scalar.dma_start(
                out=W0[:, 0, :], in_=tg.ap().rearrange("(p q) c -> p (q c)", p=P)
            )
            nc.sync.dma_start(out=CN[:, :], in_=cn.ap())
            CNB = pool.tile([P, len(BF_K) * NST], bf, tag="CNB")
            nc.sync.dma_start(out=CNB[:, :], in_=cnb.ap())

            # ---- a = tanh(x/2) ---------------------------------------------
            A = pool.tile([P, IB * C_SHARD], f32, tag="A")
            nc.scalar.activation(
                A[:, :], X[:, :], mybir.ActivationFunctionType.Tanh, scale=0.5
            )

            # ---- masked power tiles W_k[p, s, ibc] = mask_s * a^k ----------
            # k = 0..2 in fp32 (dominant coefficients), k = 3..6 in bf16
            # (tiny contributions -> rounding is far below fp32 noise floor).
            # The two squares run on the otherwise-idle scalar engine.
            W1 = pool.tile(shp, f32, tag="W1")
            W2 = pool.tile(shp, f32, tag="W2")
            W3b = pool.tile(shp, bf, tag="W3b")
            W4b = pool.tile(shp, bf, tag="W4b")
            W5b = pool.tile(shp, bf, tag="W5b")
            W6b = pool.tile(shp, bf, tag="W6b")
            nc.vector.tensor_scalar(W0[:, 1, :], W0[:, 0, :], -1.0, 1.0,
                                    op0=mult, op1=add)                  # 1 - t
            nc.vector.tensor_mul(W1[:, 0, :], W0[:, 0, :], A[:, :])
            nc.vector.tensor_mul(W1[:, 1, :], W0[:, 1, :], A[:, :])
            nc.vector.tensor_mul(W2[:, :, :], W1[:, :, :], W1[:, :, :])
            nc.vector.tensor_mul(W3b[:, :, :], W1[:, :, :], W2[:, :, :])
            nc.vector.tensor_mul(W6b[:, :, :], W3b[:, :, :], W3b[:, :, :])
            nc.scalar.activation(W4b[:, :, :], W2[:, :, :],
                                 mybir.ActivationFunctionType.Square)
            nc.vector.tensor_mul(W5b[:, :, :], W2[:, :, :], W3b[:, :, :])

            # ---- PE: batch-sum + coefficient combination -------------------
            # PSA [14, 256] accumulates the wide moments (k=0 fp32, k=3..6
            # bf16).  k=1,2 are ib-prefolded on DVE after the chain (R1/R2,
            # [128, 32]) so their fp32 LOW/HIGH matmuls stream 32 columns
            # instead of 256; they accumulate in PSB [14, 32].
            PSA = pps.tile([NST, 2 * IB * C_SHARD], f32, tag="PSA")
            PSB = pps.tile([NST, 2 * C_SHARD], f32, tag="PSB")
            mm_plan = [
                (W0, CN[:, 0 * NST:1 * NST]),
                (W3b, CNB[:, 0 * NST:1 * NST]),
                (W6b, CNB[:, 3 * NST:4 * NST]),
                (W4b, CNB[:, 1 * NST:2 * NST]),
                (W5b, CNB[:, 2 * NST:3 * NST]),
            ]
            for k, (wk, lhsT) in enumerate(mm_plan):
                nc.tensor.matmul(
                    PSA[:, :], lhsT, wk[:, :, :],
                    start=(k == 0), stop=(k == len(mm_plan) - 1),
                )
            R1 = pool.tile([P, 2 * C_SHARD], f32, tag="R1")
            R2 = pool.tile([P, 2 * C_SHARD], f32, tag="R2")
            nc.vector.reduce_sum(
                R1[:, :].rearrange("p (s c) -> p s c", s=2),
                W1[:, :, :].rearrange("p s (ib c) -> p s c ib", ib=IB),
                axis=mybir.AxisListType.X,
            )
            nc.vector.reduce_sum(
                R2[:, :].rearrange("p (s c) -> p s c", s=2),
                W2[:, :, :].rearrange("p s (ib c) -> p s c ib", ib=IB),
                axis=mybir.AxisListType.X,
            )
            nc.tensor.matmul(PSB[:, :], CN[:, 1 * NST:2 * NST], R1[:, :],
                             start=True, stop=False)
            nc.tensor.matmul(PSB[:, :], CN[:, 2 * NST:3 * NST], R2[:, :],
                             start=False, stop=True)

            # ---- fold the ib axis of PSA, merge PSB: SBF[j, s*16+c] --------
            SB = pool.tile([NST, 2 * C_SHARD], f32, tag="SB")
            nc.vector.reduce_sum(
                SB[:, :].rearrange("p (s c) -> p s c", s=2),
                PSA[:, :].rearrange("p (s ib c) -> p s c ib", s=2, ib=IB),
                axis=mybir.AxisListType.X,
            )
            SBF = pool.tile([NST, 2 * C_SHARD], f32, tag="SBF")
            nc.vector.tensor_add(SBF[:, :], SB[:, :], PSB[:, :])
            # rows 0..6 cols 0:16   = H[m,c] = sum_n Bm[m,n] Sp[n,c]
            # rows 7..13            = [Sp[k] | Sn[k]]

            # ---- relocate raw rows 7..13 to partitions 0..6 (matmul moves
            # partitions; DVE slices must start at partition 0/32/64/96) ----
            RAW = pps.tile([NMOM, 2 * C_SHARD], f32, tag="RAW")
            nc.tensor.matmul(
                RAW[:, :], CN[0:NST, SEL_COL:SEL_COL + NMOM], SBF[:, :],
                start=True, stop=True,
            )

            # ---- num[c] = sum_m Sn[m,c] * H[m,c] ---------------------------
            G = pool.tile([NMOM, C_SHARD], f32, tag="G")
            nc.vector.tensor_mul(
                G[:, :], RAW[:, C_SHARD:2 * C_SHARD], SBF[0:NMOM, 0:C_SHARD]
            )
            NUM = pps.tile([1, C_SHARD], f32, tag="NUM")
            nc.tensor.matmul(
                NUM[:, :], CN[0:NMOM, ONES_COL:ONES_COL + 1], G[:, :],
                start=True, stop=True,
            )

            # ---- per-class mean + validity ---------------------------------
            # n_neg = B - n_pos exactly; work with cntneg = (Sp0 - B)*Sp0 =
            # -cnt so each op reads PSUM at most once (no two-PSUM operands).
            # The validity channel is stored negated; the host flips the sign.
            RES = pool.tile([1, 2, C_SHARD], f32, tag="RES")
            nneg = pool.tile([1, C_SHARD], f32, tag="nneg")
            nc.vector.tensor_scalar(nneg[:, :], RAW[0:1, 0:C_SHARD],
                                    -1.0, float(B_FULL), op0=mult, op1=add)
            cnt = pool.tile([1, C_SHARD], f32, tag="cnt")
            nc.vector.tensor_mul(cnt[:, :], nneg[:, :], RAW[0:1, 0:C_SHARD])
            nc.vector.tensor_scalar(RES[:, 1, :], cnt[:, :], 0.5, None, op0=is_gt)
            safe = pool.tile([1, C_SHARD], f32, tag="safe")
            nc.vector.tensor_scalar_max(safe[:, :], cnt[:, :], 1.0)
            rec = pool.tile([1, C_SHARD], f32, tag="rec")
            nc.vector.reciprocal(rec[:, :], safe[:, :])
            mask2 = pool.tile([1, C_SHARD], f32, tag="mask2")
            nc.vector.tensor_mul(mask2[:, :], rec[:, :], RES[:, 1, :])
            nc.vector.tensor_mul(RES[:, 0, :], NUM[:, :], mask2[:, :])

            OUT = pool.tile([1, 2], f32, tag="OUT")
            nc.vector.reduce_sum(OUT[:, :], RES[:, :, :], axis=mybir.AxisListType.X)
            nc.sync.dma_start(out=out_d.ap(), in_=OUT[:, :])

    nc.compile()
    return nc


_CACHE = {}


def _compiled():
    if "nc" not in _CACHE:
        _CACHE["nc"] = build_bass()
    return _CACHE["nc"]


def make_in_maps(logits, targets):
    cn, cnb = _host_consts()
    logits = np.ascontiguousarray(logits, dtype=np.float32)
    targets = np.ascontiguousarray(targets, dtype=np.float32)
    in_maps = []
    for k in range(N_CORES):
        sl = slice(k * C_SHARD, (k + 1) * C_SHARD)
        in_maps.append({
            "logits": np.ascontiguousarray(logits[:, sl]),
            "targets": np.ascontiguousarray(targets[:, sl]),
            "cn": cn,
            "cnb": cnb,
        })
    return in_maps


def combine_outputs(core_outs):
    """core_outs: list of [1,2] arrays -> scalar loss (matches reference)."""
    f32 = np.float32
    parts = np.stack([np.asarray(o, f32).reshape(2) for o in core_outs])
    sums = parts[:, 0].sum(dtype=f32)
    vc = parts[:, 1].sum(dtype=f32)
    if vc > 0:
        loss = f32(sums / max(vc, f32(1.0)))
    else:
        loss = f32(0.0)
    return np.asarray(loss, dtype=np.float32)


def kernel(logits, targets):
    nc = _compiled()
    in_maps = make_in_maps(logits, targets)
    res = bass_utils.run_bass_kernel_spmd(nc, in_maps, core_ids=list(range(N_CORES)))
    return combine_outputs([r["out"] for r in res.results])


# revision 39
# speedup vs baseline: 1.2196x; 1.0119x over previous
"""AUCM loss (pairwise softplus AUC surrogate) Trainium2 kernel.

Reference, for logits/targets [B=1024, C=128]:
    probs = sigmoid(logits)
    num[c] = sum_{i,j} softplus(p_j - p_i) * pos[i,c] * neg[j,c]
    loss   = masked mean over classes of num[c] / (n_pos[c]*n_neg[c])

Direct evaluation is O(B^2 C) = 134M softplus terms.  Since probs in (0,1),
the pairwise argument lies in (-1,1) where softplus is analytic (nearest
complex singularity at +-i*pi), so a degree-6 Chebyshev fit of softplus on
[-1,1] (max err 3.3e-7) turns the pairwise sum into per-class weighted power
sums ("moments") via the binomial expansion:

    num[c] = sum_{m+n<=6} Bm[m,n] * Sn[m,c] * Sp[n,c]
    Sp[n,c] = sum_i pos[i,c] a_i^n,  Sn[m,c] = sum_j neg[j,c] a_j^m

with a_i = tanh(logits_i/2) = 2*(probs_i - 0.5) computed in ONE activation op
(coefficients pre-scaled by 2^-k on the host).  O(B C D) work.

Sharding: data-parallel over the class axis (16 classes/core, batch
replicated, per the pairwise structure).  Each core returns its partial
(sum of per-class means, count of valid classes); the host unshard step sums
the 8 partial pairs and forms the final scalar exactly as the reference does.

Per-core dataflow ([128p, 128f] tile, partition p holds batch rows 8p..8p+7):
  - DVE builds the masked power tiles W_k[p, s, ibc] = mask_s * a^k via a
    square/product chain; k=0..2 in fp32 (dominant coefficients), k=3..6 in
    bf16 (tiny contributions); one square runs on the scalar engine.
  - PE does the batch reduction AND the coefficient combination in
    accumulating matmul groups: the stationary for moment k is [128, 14]
    with columns j<7 = Bm[j,k] (accumulates H = Bm @ Sp directly) and
    columns j>=7 = one-hot k (collects raw moments).  k=1,2 are ib-prefolded
    on DVE so their fp32 LOW/HIGH matmuls stream 32 columns instead of 256.
    Dummy warmup matmuls during the input-DMA window keep the PE HAM clock
    gate open.
  - One DVE segmented reduce folds the 8-way batch axis, a tiny selection
    matmul relocates raw-moment rows to partition 0 (DVE slices must start
    at partition 0/32/64/96).
  - Tail: G = Sn (.) H, num = ones @ G, per-class mean + validity masking,
    and a [1,2] result (sum of means, valid count) DMA'd out.
"""

import os
import sys
from math import comb

import numpy as np

for _p in ("/opt/trn_rl_repo", "/root/.axon_site/_ro/trn_rl_repo"):
    if os.path.isdir(_p) and _p not in sys.path:
        sys.path.append(_p)

import ml_dtypes

import concourse.bacc as bacc
import concourse.mybir as mybir
import concourse.tile as tile
from concourse import bass_utils

B_FULL, C_FULL = 1024, 128
N_CORES = 8
C_SHARD = C_FULL // N_CORES          # 16 classes per core
P = 128                              # partitions
IB = B_FULL // P                     # 8 batch rows folded per partition
DEG = 6
NMOM = DEG + 1                       # 7 moments (k = 0..6)
NST = 2 * NMOM                       # stationary columns (H part + raw part)
ONES_COL = NMOM * NST                # all-ones column (final sum lhsT)
SEL_COL = ONES_COL + 1               # 7-wide row-selection block (rows 7..13)
CN_COLS = SEL_COL + NMOM + 1         # + pad
BF_K = (3, 4, 5, 6)                  # moments with bf16 data + stationaries

# Degree-6 Chebyshev fit of softplus on [-1, 1] (max err 3.3e-7), monomial.
A_COEF = np.array(
    [0.6931471805599451, 0.5, 0.12499748720039783, 0.0,
     -0.005188028447445448, 0.0, 0.0003053804886608954],
    dtype=np.float64,
)


def _host_consts():
    # moments are of a = tanh(x/2) = 2*(p - 0.5); rescale poly coeffs by 2^-k
    alpha = A_COEF / (2.0 ** np.arange(NMOM))
    bm = np.zeros((NMOM, NMOM))
    for m in range(NMOM):
        for n in range(NMOM - m):
            bm[m, n] = alpha[m + n] * comb(m + n, m) * ((-1.0) ** n)
    row = np.zeros(CN_COLS, np.float32)
    for k in range(NMOM):
        row[k * NST:k * NST + NMOM] = bm[:, k]       # H-part: col j = Bm[j, k]
        row[k * NST + NMOM + k] = 1.0                # raw part: one-hot k
    row[ONES_COL] = 1.0                              # ones column (final sum)
    cn = np.ascontiguousarray(np.broadcast_to(row, (P, CN_COLS)), np.float32)
    # row-selection block: lhsT [14, 7] picking rows 7..13 down to 0..6
    for m in range(NMOM):
        cn[NMOM + m, SEL_COL + m] = 1.0
    # bf16 stationaries for the small-contribution moments k in BF_K
    rowb = np.zeros((len(BF_K), NST), np.float32)
    for i, k in enumerate(BF_K):
        rowb[i, :NMOM] = bm[:, k]
        rowb[i, NMOM + k] = 1.0
    cnb = np.ascontiguousarray(
        np.broadcast_to(rowb.reshape(1, -1), (P, len(BF_K) * NST))
    ).astype(ml_dtypes.bfloat16)
    return cn, cnb


def build_bass():
    f32 = mybir.dt.float32
    nc = bacc.Bacc("TRN2", target_bir_lowering=False, debug=False)

    bf = mybir.dt.bfloat16
    lg = nc.dram_tensor("logits", [B_FULL, C_SHARD], f32, kind="ExternalInput")
    tg = nc.dram_tensor("targets", [B_FULL, C_SHARD], f32, kind="ExternalInput")
    cn = nc.dram_tensor("cn", [P, CN_COLS], f32, kind="ExternalInput")
    cnb = nc.dram_tensor("cnb", [P, len(BF_K) * NST], bf, kind="ExternalInput")
    out_d = nc.dram_tensor("out", [1, 2], f32, kind="ExternalOutput")

    mult = mybir.AluOpType.mult
    add = mybir.AluOpType.add
    is_gt = mybir.AluOpType.is_gt

    with tile.TileContext(nc) as tc:
        with (
            tc.tile_pool(name="sb", bufs=1) as pool,
            tc.tile_pool(name="ps", bufs=1, space="PSUM") as pps,
        ):
            # ---- PE warmup: dummy matmuls during the input-DMA window so
            # the HAM clock gate is at 2.4 GHz when the real matmuls arrive.
            # Their PSUM tile is never read; contents are irrelevant.
            WU = pool.tile([P, 256], mybir.dt.bfloat16, tag="WU")
            nc.gpsimd.memset(WU[:, :], 0.0)
            DPS = pps.tile([1, 256], f32, tag="DPS")
            for _ in range(15):
                nc.tensor.matmul(DPS[:, :], WU[:, 0:1], WU[:, :],
                                 start=True, stop=True)

            # ---- inputs -> SBUF (contiguous loads, two HWDGE rings) --------
            # targets land directly in W0's pos half (saves a copy op)
            shp = [P, 2, IB * C_SHARD]
            W0 = pool.tile(shp, f32, tag="W0")
            X = pool.tile([P, IB * C_SHARD], f32, tag="X")
            CN = pool.tile([P, CN_COLS], f32, tag="CN")
            nc.sync.dma_start(
                out=X[:, :], in_=lg.ap().rearrange("(p q) c -> p (q c)", p=P)
            )
            nc.scalar.dma_start(
                out=W0[:, 0, :], in_=tg.ap().rearrange("(p q) c -> p (q c)", p=P)
            )
            nc.sync.dma_start(out=CN[:, :], in_=cn.ap())
            CNB = pool.tile([P, len(BF_K) * NST], bf, tag="CNB")
            nc.sync.dma_start(out=CNB[:, :], in_=cnb.ap())

            # ---- a = tanh(x/2) ---------------------------------------------
            A = pool.tile([P, IB * C_SHARD], f32, tag="A")
            nc.scalar.activation(
                A[:, :], X[:, :], mybir.ActivationFunctionType.Tanh, scale=0.5
            )

            # ---- masked power tiles W_k[p, s, ibc] = mask_s * a^k ----------
            # k = 0..2 in fp32 (dominant coefficients), k = 3..6 in bf16
            # (tiny contributions -> rounding is far below fp32 noise floor).
            # The two squares run on the otherwise-idle scalar engine.
            W1 = pool.tile(shp, f32, tag="W1")
            W2 = pool.tile(shp, f32, tag="W2")
            W3b = pool.tile(shp, bf, tag="W3b")
            W4b = pool.tile(shp, bf, tag="W4b")
            W5b = pool.tile(shp, bf, tag="W5b")
            W6b = pool.tile(shp, bf, tag="W6b")
            nc.vector.tensor_scalar(W0[:, 1, :], W0[:, 0, :], -1.0, 1.0,
                                    op0=mult, op1=add)                  # 1 - t
            nc.vector.tensor_mul(W1[:, 0, :], W0[:, 0, :], A[:, :])
            nc.vector.tensor_mul(W1[:, 1, :], W0[:, 1, :], A[:, :])
            nc.vector.tensor_mul(W2[:, :, :], W1[:, :, :], W1[:, :, :])
            nc.vector.tensor_mul(W3b[:, :, :], W1[:, :, :], W2[:, :, :])
            nc.vector.tensor_mul(W6b[:, :, :], W3b[:, :, :], W3b[:, :, :])
            nc.scalar.activation(W4b[:, :, :], W2[:, :, :],
                                 mybir.ActivationFunctionType.Square)
            nc.vector.tensor_mul(W5b[:, :, :], W2[:, :, :], W3b[:, :, :])

            # ---- PE: batch-sum + coefficient combination -------------------
            # PSA [14, 256] accumulates the wide moments (k=0 fp32, k=3..6
            # bf16).  k=1,2 are ib-prefolded on DVE after the chain (R1/R2,
            # [128, 32]) so their fp32 LOW/HIGH matmuls stream 32 columns
            # instead of 256; they accumulate in PSB [14, 32].
            PSA = pps.tile([NST, 2 * IB * C_SHARD], f32, tag="PSA")
            PSB = pps.tile([NST, 2 * C_SHARD], f32, tag="PSB")
            mm_plan = [
                (W0, CN[:, 0 * NST:1 * NST]),
                (W3b, CNB[:, 0 * NST:1 * NST]),
                (W6b, CNB[:, 3 * NST:4 * NST]),
                (W4b, CNB[:, 1 * NST:2 * NST]),
                (W5b, CNB[:, 2 * NST:3 * NST]),
            ]
            for k, (wk, lhsT) in enumerate(mm_plan):
                nc.tensor.matmul(
                    PSA[:, :], lhsT, wk[:, :, :],
                    start=(k == 0), stop=(k == len(mm_plan) - 1),
                )
            R1 = pool.tile([P, 2 * C_SHARD], f32, tag="R1")
            R2 = pool.tile([P, 2 * C_SHARD], f32, tag="R2")
            nc.vector.reduce_sum(
                R1[:, :].rearrange("p (s c) -> p s c", s=2),
                W1[:, :, :].rearrange("p s (ib c) -> p s c ib", ib=IB),
                axis=mybir.AxisListType.X,
            )
            nc.vector.reduce_sum(
                R2[:, :].rearrange("p (s c) -> p s c", s=2),
                W2[:, :, :].rearrange("p s (ib c) -> p s c ib", ib=IB),
                axis=mybir.AxisListType.X,
            )
            nc.tensor.matmul(PSB[:, :], CN[:, 1 * NST:2 * NST], R1[:, :],
                             start=True, stop=False)
            nc.tensor.matmul(PSB[:, :], CN[:, 2 * NST:3 * NST], R2[:, :],
                             start=False, stop=True)

            # ---- fold the ib axis of PSA, merge PSB: SBF[j, s*16+c] --------
            SB = pool.tile([NST, 2 * C_SHARD], f32, tag="SB")
            nc.vector.reduce_sum(
                SB[:, :].rearrange("p (s c) -> p s c", s=2),
                PSA[:, :].rearrange("p (s ib c) -> p s c ib", s=2, ib=IB),
                axis=mybir.AxisListType.X,
            )
            SBF = pool.tile([NST, 2 * C_SHARD], f32, tag="SBF")
            nc.vector.tensor_add(SBF[:, :], SB[:, :], PSB[:, :])
            # rows 0..6 cols 0:16   = H[m,c] = sum_n Bm[m,n] Sp[n,c]
            # rows 7..13            = [Sp[k] | Sn[k]]

            # ---- relocate raw rows 7..13 to partitions 0..6 (matmul moves
            # partitions; DVE slices must start at partition 0/32/64/96) ----
            RAW = pps.tile([NMOM, 2 * C_SHARD], f32, tag="RAW")
            nc.tensor.matmul(
                RAW[:, :], CN[0:NST, SEL_COL:SEL_COL + NMOM], SBF[:, :],
                start=True, stop=True,
            )

            # ---- num[c] = sum_m Sn[m,c] * H[m,c] ---------------------------
            G = pool.tile([NMOM, C_SHARD], f32, tag="G")
            nc.vector.tensor_mul(
                G[:, :], RAW[:, C_SHARD:2 * C_SHARD], SBF[0:NMOM, 0:C_SHARD]
            )
            NUM = pps.tile([1, C_SHARD], f32, tag="NUM")
            nc.tensor.matmul(
                NUM[:, :], CN[0:NMOM, ONES_COL:ONES_COL + 1], G[:, :],
                start=True, stop=True,
            )

            # ---- per-class mean + validity ---------------------------------
            # n_neg = B - n_pos exactly; work with cntneg = (Sp0 - B)*Sp0 =
            # -cnt so each op reads PSUM at most once (no two-PSUM operands).
            # The validity channel is stored negated; the host flips the sign.
            RES = pool.tile([1, 2, C_SHARD], f32, tag="RES")
            nneg = pool.tile([1, C_SHARD], f32, tag="nneg")
            nc.vector.tensor_scalar(nneg[:, :], RAW[0:1, 0:C_SHARD],
                                    -1.0, float(B_FULL), op0=mult, op1=add)
            cnt = pool.tile([1, C_SHARD], f32, tag="cnt")
            nc.vector.tensor_mul(cnt[:, :], nneg[:, :], RAW[0:1, 0:C_SHARD])
            # For an invalid class every moment partial is an exact 0, so
            # num == 0 exactly and num/max(cnt,1) is already the masked
            # per-class mean -- no valid-mask multiply needed on this path.
            nc.vector.tensor_scalar(RES[:, 1, :], cnt[:, :], 0.5, None, op0=is_gt)
            safe = pool.tile([1, C_SHARD], f32, tag="safe")
            nc.vector.tensor_scalar_max(safe[:, :], cnt[:, :], 1.0)
            rec = pool.tile([1, C_SHARD], f32, tag="rec")
            nc.vector.reciprocal(rec[:, :], safe[:, :])
            nc.vector.tensor_mul(RES[:, 0, :], NUM[:, :], rec[:, :])

            OUT = pool.tile([1, 2], f32, tag="OUT")
            nc.vector.reduce_sum(OUT[:, :], RES[:, :, :], axis=mybir.AxisListType.X)
            nc.sync.dma_start(out=out_d.ap(), in_=OUT[:, :])

    nc.compile()
    return nc


_CACHE = {}


def _compiled():
    if "nc" not in _CACHE:
        _CACHE["nc"] = build_bass()
    return _CACHE["nc"]


def make_in_maps(logits, targets):
    cn, cnb = _host_consts()
    logits = np.ascontiguousarray(logits, dtype=np.float32)
    targets = np.ascontiguousarray(targets, dtype=np.float32)
    in_maps = []
    for k in range(N_CORES):
        sl = slice(k * C_SHARD, (k + 1) * C_SHARD)
        in_maps.append({
            "logits": np.ascontiguousarray(logits[:, sl]),
            "targets": np.ascontiguousarray(targets[:, sl]),
            "cn": cn,
            "cnb": cnb,
        })
    return in_maps


def combine_outputs(core_outs):
    """core_outs: list of [1,2] arrays -> scalar loss (matches reference)."""
    f32 = np.float32
    parts = np.stack([np.asarray(o, f32).reshape(2) for o in core_outs])
    sums = parts[:, 0].sum(dtype=f32)
    vc = parts[:, 1].sum(dtype=f32)
    if vc > 0:
        loss = f32(sums / max(vc, f32(1.0)))
    else:
        loss = f32(0.0)
    return np.asarray(loss, dtype=np.float32)


def kernel(logits, targets):
    nc = _compiled()
    in_maps = make_in_maps(logits, targets)
    res = bass_utils.run_bass_kernel_spmd(nc, in_maps, core_ids=list(range(N_CORES)))
    return combine_outputs([r["out"] for r in res.results])
